# revision 1
# baseline (speedup 1.0000x reference)
import sys
sys.path.insert(0, '/opt/trn_rl_repo')
import numpy as np
import ml_dtypes

from concourse import bacc, bass, tile, mybir
from concourse.bass_utils import run_bass_kernel_spmd

BF16 = ml_dtypes.bfloat16
F32 = mybir.dt.float32
BF = mybir.dt.bfloat16
AF = mybir.ActivationFunctionType

NC = 8
N, M, N0 = 50000, 12, 5000
AFL, NBR, ORIG, HF, NG = 64, 64, 200, 128, 3
EPS = 1e-5
NLOC = N // NC            # 6250
COLS = NLOC * M           # 75000
GP = 2016                 # cols per psum group (4 banks x 504)
NGRP = 38                 # 37 full + 408 partial
LASTW = COLS - 37 * GP    # 408
APAD = 38 * 168           # 6384 padded atoms per core
CLOC = N0 // NC           # 625

_cache = {}


def _grp_width(g):
    return GP if g < 37 else LASTW


def _build_embed():
    nc = bacc.Bacc(None, target_bir_lowering=False)
    ta = nc.dram_tensor("oa", [128, APAD], BF, kind="ExternalInput")
    tb = nc.dram_tensor("ob", [72, APAD], BF, kind="ExternalInput")
    twa = nc.dram_tensor("wa", [128, 64], BF, kind="ExternalInput")
    twb = nc.dram_tensor("wb", [72, 64], BF, kind="ExternalInput")
    tbi = nc.dram_tensor("eb", [64, 1], F32, kind="ExternalInput")
    tout = nc.dram_tensor("atom", [64, APAD], F32, kind="ExternalOutput")
    with tile.TileContext(nc) as tc:
        with tc.tile_pool(name="p", bufs=3) as pool, \
             tc.tile_pool(name="w", bufs=1) as wp, \
             tc.tile_pool(name="ps", bufs=4, space="PSUM") as pp:
            wa = wp.tile([128, 64], BF, name="wa", tag="wa"); nc.sync.dma_start(wa[:], twa[:])
            wb = wp.tile([72, 64], BF, name="wb", tag="wb"); nc.sync.dma_start(wb[:], twb[:])
            eb = wp.tile([64, 1], F32, name="eb", tag="eb"); nc.sync.dma_start(eb[:], tbi[:])
            nchunk = APAD // 336  # 19 chunks of 336
            for c in range(nchunk):
                s = slice(c * 336, (c + 1) * 336)
                xa = pool.tile([128, 336], BF, name="xa", tag="xa")
                xb = pool.tile([72, 336], BF, name="xb", tag="xb")
                nc.sync.dma_start(xa[:], ta[:, s])
                nc.sync.dma_start(xb[:], tb[:, s])
                ps = pp.tile([64, 336], F32, name="ps", tag="ps")
                nc.tensor.matmul(ps[:], wa[:], xa[:], start=True, stop=False)
                nc.tensor.matmul(ps[:], wb[:], xb[:], start=False, stop=True)
                ot = pool.tile([64, 336], F32, name="ot", tag="ot")
                nc.scalar.activation(ot[:], ps[:], AF.Identity, bias=eb[:], scale=1.0)
                nc.sync.dma_start(tout[:, s], ot[:])
    nc.compile()
    return nc


def _build_layer():
    nc = bacc.Bacc(None, target_bir_lowering=False)
    tgs = nc.dram_tensor("gs", [128, COLS], BF, kind="ExternalInput")
    tnb = nc.dram_tensor("nb", [64, COLS], BF, kind="ExternalInput")
    tat = nc.dram_tensor("atin", [64, APAD], F32, kind="ExternalInput")
    twsn = nc.dram_tensor("wsn", [128, 128], BF, kind="ExternalInput")
    twe = nc.dram_tensor("we", [64, 128], BF, kind="ExternalInput")
    tb2 = nc.dram_tensor("b2", [128, 1], F32, kind="ExternalInput")
    tg2 = nc.dram_tensor("g2", [128, 1], F32, kind="ExternalInput")
    tbb2 = nc.dram_tensor("bb2", [128, 1], F32, kind="ExternalInput")
    tg1 = nc.dram_tensor("g1", [64, 1], F32, kind="ExternalInput")
    tbb1 = nc.dram_tensor("bb1", [64, 1], F32, kind="ExternalInput")
    tout = nc.dram_tensor("atout", [64, APAD], F32, kind="ExternalOutput")
    rg = [list(range(NC))]
    with tile.TileContext(nc) as tc:
        with tc.tile_pool(name="p", bufs=4) as pool, \
             tc.tile_pool(name="w", bufs=1) as wp, \
             tc.tile_pool(name="st", bufs=1) as sp, \
             tc.tile_pool(name="z", bufs=2) as zp, \
             tc.tile_pool(name="d", bufs=1, space="DRAM") as dp, \
             tc.tile_pool(name="ps", bufs=2, space="PSUM") as pp:
            wsn = wp.tile([128, 128], BF, name="wsn", tag="wsn"); nc.sync.dma_start(wsn[:], twsn[:])
            we = wp.tile([64, 128], BF, name="we", tag="we"); nc.sync.dma_start(we[:], twe[:])
            b2 = wp.tile([128, 1], F32, name="b2", tag="b2"); nc.sync.dma_start(b2[:], tb2[:])
            g2 = wp.tile([128, 1], F32, name="g2", tag="g2"); nc.sync.dma_start(g2[:], tg2[:])
            bb2 = wp.tile([128, 1], F32, name="bb2", tag="bb2"); nc.sync.dma_start(bb2[:], tbb2[:])
            g1 = wp.tile([64, 1], F32, name="g1", tag="g1"); nc.sync.dma_start(g1[:], tg1[:])
            bb1 = wp.tile([64, 1], F32, name="bb1", tag="bb1"); nc.sync.dma_start(bb1[:], tbb1[:])
            atin = wp.tile([64, APAD], F32, name="atin", tag="atin"); nc.sync.dma_start(atin[:], tat[:])

            sumac = sp.tile([128, NGRP], F32, name="sumac", tag="sumac")
            sqac = sp.tile([128, NGRP], F32, name="sqac", tag="sqac")

            def mk_psum(g):
                w = _grp_width(g)
                nb_ = min(4, (w + 503) // 504)
                ps = pp.tile([128, 4, 512], F32, name="ps", tag="ps")
                for k in range(nb_):
                    c0 = g * GP + k * 504
                    cw = min(504, g * GP + w - c0)
                    gst = pool.tile([128, 504], BF, name="gst", tag="gst")
                    nbt = pool.tile([64, 504], BF, name="nbt", tag="nbt")
                    nc.sync.dma_start(gst[:, 0:cw], tgs[:, c0:c0 + cw])
                    nc.sync.dma_start(nbt[:, 0:cw], tnb[:, c0:c0 + cw])
                    nc.tensor.matmul(ps[:, k, 0:cw], wsn[:], gst[:, 0:cw],
                                     start=True, stop=False)
                    nc.tensor.matmul(ps[:, k, 0:cw], we[:], nbt[:, 0:cw],
                                     start=False, stop=True)
                return ps

            # ---- pass A: stats ----
            for g in range(NGRP):
                w = _grp_width(g)
                ps = mk_psum(g)
                ap = ps[:, 0:1, 0:w] if w < GP else ps[:, :, 0:504]
                d1 = pool.tile([128, GP], BF, name="dump", tag="dump")
                nc.scalar.activation(d1[:, 0:w].rearrange("p (b c) -> p b c", b=ap.shape[1]),
                                     ap, AF.Copy, accum_out=sumac[:, g:g + 1])
                d2 = pool.tile([128, GP], BF, name="dump", tag="dump")
                nc.scalar.activation(d2[:, 0:w].rearrange("p (b c) -> p b c", b=ap.shape[1]),
                                     ap, AF.Square, accum_out=sqac[:, g:g + 1])

            st = sp.tile([128, 2], F32, name="st", tag="st")
            nc.vector.tensor_reduce(st[:, 0:1], sumac[:], mybir.AxisListType.X, mybir.AluOpType.add)
            nc.vector.tensor_reduce(st[:, 1:2], sqac[:], mybir.AxisListType.X, mybir.AluOpType.add)
            bnin = dp.tile([128, 2], F32, name="bnin", tag="bnin")
            bnout = dp.tile([128, 2], F32, name="bnout", tag="bnout")
            nc.gpsimd.dma_start(bnin[:], st[:])
            nc.gpsimd.collective_compute("AllReduce", mybir.AluOpType.add,
                                         ins=[bnin.opt()], outs=[bnout.opt()],
                                         replica_groups=rg)
            stt = sp.tile([128, 2], F32, name="stt", tag="stt")
            nc.sync.dma_start(stt[:], bnout[:])
            inv = 1.0 / (N * M)
            mean = sp.tile([128, 1], F32, name="mean", tag="mean")
            nc.vector.tensor_scalar_mul(mean[:], stt[:, 0:1], inv)
            ex2 = sp.tile([128, 1], F32, name="ex2", tag="ex2")
            nc.vector.tensor_scalar_mul(ex2[:], stt[:, 1:2], inv)
            var = sp.tile([128, 1], F32, name="var", tag="var")
            nc.vector.tensor_tensor(var[:], mean[:], mean[:], mybir.AluOpType.mult)
            nc.vector.tensor_tensor(var[:], ex2[:], var[:], mybir.AluOpType.subtract)
            sd = sp.tile([128, 1], F32, name="sd", tag="sd")
            nc.vector.tensor_scalar_add(var[:], var[:], EPS)
            nc.scalar.activation(sd[:], var[:], AF.Sqrt)
            rstd = sp.tile([128, 1], F32, name="rstd", tag="rstd")
            nc.vector.reciprocal(rstd[:], sd[:])
            sA = sp.tile([128, 1], F32, name="sA", tag="sA")
            nc.vector.tensor_tensor(sA[:], g2[:], rstd[:], mybir.AluOpType.mult)
            tA = sp.tile([128, 1], F32, name="tA", tag="tA")
            nc.vector.tensor_tensor(tA[:], mean[:], sA[:], mybir.AluOpType.mult)
            nc.vector.tensor_tensor(tA[:], bb2[:], tA[:], mybir.AluOpType.subtract)
            # msg bias b2 cancels under batchnorm (shift invariance)
            tA2 = tA

            # ---- pass B ----
            stk = sp.tile([128, 19, 4, 42], F32, name="stk", tag="stk")
            for pr in range(19):
                zF = zp.tile([128, 4, 504], BF, name="zF", tag="zF")
                zC = zp.tile([128, 4, 504], BF, name="zC", tag="zC")
                for half, g in ((0, 2 * pr), (1, 2 * pr + 1)):
                    w = _grp_width(g)
                    ps = mk_psum(g)
                    nb_ = 1 if w < GP else 4
                    cw = w if w < GP else 504
                    if w < GP:
                        nc.vector.memset(zF[64:128, :, :], 0.0)
                        nc.vector.memset(zC[64:128, :, :], 0.0)
                    if half == 0:
                        nc.scalar.activation(zF[0:64, 0:nb_, 0:cw], ps[0:64, 0:nb_, 0:cw],
                                             AF.Sigmoid, scale=sA[0:64, :], bias=tA2[0:64, :])
                        tmpC = zp.tile([128, 4, 504], BF, name="tmpC", tag="tmpC")
                        nc.scalar.activation(tmpC[64:128, 0:nb_, 0:cw], ps[64:128, 0:nb_, 0:cw],
                                             AF.Exp, scale=sA[64:128, :], bias=tA2[64:128, :])
                        nc.scalar.activation(tmpC[64:128, 0:nb_, 0:cw], tmpC[64:128, 0:nb_, 0:cw],
                                             AF.Ln, bias=1.0, scale=1.0)
                        nc.sync.dma_start(zC[0:64, 0:nb_, 0:cw], tmpC[64:128, 0:nb_, 0:cw])
                    else:
                        tmpF = zp.tile([64, 4, 504], BF, name="tmpF", tag="tmpF")
                        nc.scalar.activation(tmpF[0:64, 0:nb_, 0:cw], ps[0:64, 0:nb_, 0:cw],
                                             AF.Sigmoid, scale=sA[0:64, :], bias=tA2[0:64, :])
                        nc.sync.dma_start(zF[64:128, 0:nb_, 0:cw], tmpF[0:64, 0:nb_, 0:cw])
                        nc.scalar.activation(zC[64:128, 0:nb_, 0:cw], ps[64:128, 0:nb_, 0:cw],
                                             AF.Exp, scale=sA[64:128, :], bias=tA2[64:128, :])
                        nc.scalar.activation(zC[64:128, 0:nb_, 0:cw], zC[64:128, 0:nb_, 0:cw],
                                             AF.Ln, bias=1.0, scale=1.0)
                z = zp.tile([128, 4, 504], BF, name="zz", tag="zz")
                nc.vector.tensor_tensor(z[:], zF[:], zC[:], mybir.AluOpType.mult)
                zv = z[:].rearrange("p b (a m) -> p b a m", m=12)
                nc.vector.tensor_copy(stk[:, pr, :, :], zv[:, :, :, 0])
                for m in range(1, 12):
                    nc.vector.tensor_tensor(stk[:, pr, :, :], stk[:, pr, :, :],
                                            zv[:, :, :, m], mybir.AluOpType.add)

            # bn1 stats
            stkf = stk[:].rearrange("p g b a -> p (g b a)")
            s1 = sp.tile([128, 2], F32, name="s1", tag="s1")
            nc.vector.tensor_reduce(s1[:, 0:1], stkf, mybir.AxisListType.X, mybir.AluOpType.add)
            d3 = sp.tile([128, 19 * 168], BF, name="d3", tag="d3")
            nc.scalar.activation(d3[:].rearrange("p (g b a) -> p g b a", g=19, b=4),
                                 stk[:], AF.Square, accum_out=s1[:, 1:2])
            bn1i = dp.tile([128, 2], F32, name="bn1i", tag="bn1i")
            bn1o = dp.tile([128, 2], F32, name="bn1o", tag="bn1o")
            nc.gpsimd.dma_start(bn1i[:], s1[:])
            nc.gpsimd.collective_compute("AllReduce", mybir.AluOpType.add,
                                         ins=[bn1i.opt()], outs=[bn1o.opt()],
                                         replica_groups=rg)
            t1a = sp.tile([64, 2], F32, name="t1a", tag="t1a")
            t1b = sp.tile([64, 2], F32, name="t1b", tag="t1b")
            nc.sync.dma_start(t1a[:], bn1o[0:64, :])
            nc.sync.dma_start(t1b[:], bn1o[64:128, :])
            tot = sp.tile([64, 2], F32, name="tot", tag="tot")
            nc.vector.tensor_tensor(tot[:], t1a[:], t1b[:], mybir.AluOpType.add)
            m1 = sp.tile([64, 1], F32, name="m1", tag="m1")
            nc.vector.tensor_scalar_mul(m1[:], tot[:, 0:1], 1.0 / N)
            e21 = sp.tile([64, 1], F32, name="e21", tag="e21")
            nc.vector.tensor_scalar_mul(e21[:], tot[:, 1:2], 1.0 / N)
            v1 = sp.tile([64, 1], F32, name="v1", tag="v1")
            nc.vector.tensor_tensor(v1[:], m1[:], m1[:], mybir.AluOpType.mult)
            nc.vector.tensor_tensor(v1[:], e21[:], v1[:], mybir.AluOpType.subtract)
            sd1 = sp.tile([64, 1], F32, name="sd1", tag="sd1")
            nc.vector.tensor_scalar_add(v1[:], v1[:], EPS)
            nc.scalar.activation(sd1[:], v1[:], AF.Sqrt)
            r1 = sp.tile([64, 1], F32, name="r1", tag="r1")
            nc.vector.reciprocal(r1[:], sd1[:])
            s1v = sp.tile([64, 1], F32, name="s1v", tag="s1v")
            nc.vector.tensor_tensor(s1v[:], g1[:], r1[:], mybir.AluOpType.mult)
            t1v = sp.tile([64, 1], F32, name="t1v", tag="t1v")
            nc.vector.tensor_tensor(t1v[:], m1[:], s1v[:], mybir.AluOpType.mult)
            nc.vector.tensor_tensor(t1v[:], bb1[:], t1v[:], mybir.AluOpType.subtract)

            # atom update
            stkO = sp.tile([64, 19, 4, 42], F32, name="stkO", tag="stkO")
            nc.sync.dma_start(stkO[:], stk[64:128, :, :, :])
            atN = sp.tile([64, APAD], F32, name="atN", tag="atN")
            atv = atN[:].rearrange("p (g t b a) -> p g t b a", g=19, t=2, b=4)
            aiv = atin[:].rearrange("p (g t b a) -> p g t b a", g=19, t=2, b=4)
            nc.vector.tensor_scalar(atv[:, :, 0], stk[0:64, :, :, :], s1v[:], t1v[:],
                                    op0=mybir.AluOpType.mult, op1=mybir.AluOpType.add)
            nc.vector.tensor_tensor(atv[:, :, 0], atv[:, :, 0], aiv[:, :, 0], mybir.AluOpType.add)
            nc.vector.tensor_scalar(atv[:, :, 1], stkO[:], s1v[:], t1v[:],
                                    op0=mybir.AluOpType.mult, op1=mybir.AluOpType.add)
            nc.vector.tensor_tensor(atv[:, :, 1], atv[:, :, 1], aiv[:, :, 1], mybir.AluOpType.add)
            nc.scalar.activation(atN[:], atN[:], AF.Exp)
            nc.scalar.activation(atN[:], atN[:], AF.Ln, bias=1.0, scale=1.0)
            nc.sync.dma_start(tout[:], atN[:])
    nc.compile()
    return nc


def _build_pool():
    nc = bacc.Bacc(None, target_bir_lowering=False)
    tat = nc.dram_tensor("atin", [64, APAD], F32, kind="ExternalInput")
    tw1a = nc.dram_tensor("w1a", [64, 128], BF, kind="ExternalInput")
    tw1b = nc.dram_tensor("w1b", [64, 128], BF, kind="ExternalInput")
    tb1 = nc.dram_tensor("fb1", [128, 1], F32, kind="ExternalInput")
    two = nc.dram_tensor("wo", [128, 1], BF, kind="ExternalInput")
    tout = nc.dram_tensor("out", [1, CLOC], F32, kind="ExternalOutput")
    with tile.TileContext(nc) as tc:
        with tc.tile_pool(name="p", bufs=2) as pool, \
             tc.tile_pool(name="ps", bufs=1, space="PSUM") as pp:
            at = pool.tile([64, APAD], F32, name="at", tag="at"); nc.sync.dma_start(at[:], tat[:])
            w1a = pool.tile([64, 128], BF, name="w1a", tag="w1a"); nc.sync.dma_start(w1a[:], tw1a[:])
            w1b = pool.tile([64, 128], BF, name="w1b", tag="w1b"); nc.sync.dma_start(w1b[:], tw1b[:])
            fb1 = pool.tile([128, 1], F32, name="fb1", tag="fb1"); nc.sync.dma_start(fb1[:], tb1[:])
            wo = pool.tile([128, 1], BF, name="wo", tag="wo"); nc.sync.dma_start(wo[:], two[:])
            av = at[:, 0:NLOC].rearrange("p (c t) -> p c t", t=10)
            sm = pool.tile([64, CLOC], F32, name="sm", tag="sm")
            nc.vector.tensor_reduce(sm[:], av, mybir.AxisListType.X, mybir.AluOpType.add)
            mean = pool.tile([64, CLOC], F32, name="mean", tag="mean")
            nc.vector.tensor_scalar_mul(mean[:], sm[:], 0.1)
            sq = pool.tile([64, NLOC], F32, name="sq", tag="sq")
            nc.scalar.activation(sq[:], at[:, 0:NLOC], AF.Square)
            sqs = pool.tile([64, CLOC], F32, name="sqs", tag="sqs")
            nc.vector.tensor_reduce(sqs[:],
                                    sq[:].rearrange("p (c t) -> p c t", t=10),
                                    mybir.AxisListType.X, mybir.AluOpType.add)
            m2 = pool.tile([64, CLOC], F32, name="m2", tag="m2")
            nc.vector.tensor_tensor(m2[:], mean[:], mean[:], mybir.AluOpType.mult)
            nc.vector.tensor_scalar_mul(m2[:], m2[:], 10.0)
            d = pool.tile([64, CLOC], F32, name="d", tag="d")
            nc.vector.tensor_tensor(d[:], sqs[:], m2[:], mybir.AluOpType.subtract)
            std = pool.tile([64, CLOC], F32, name="std", tag="std")
            nc.scalar.activation(std[:], d[:], AF.Sqrt, scale=1.0 / 9.0)
            cm = pool.tile([64, CLOC], BF, name="cm", tag="cm")
            nc.scalar.activation(cm[:], mean[:], AF.Exp)
            nc.scalar.activation(cm[:], cm[:], AF.Ln, bias=1.0, scale=1.0)
            cs = pool.tile([64, CLOC], BF, name="cs", tag="cs")
            nc.scalar.activation(cs[:], std[:], AF.Exp)
            nc.scalar.activation(cs[:], cs[:], AF.Ln, bias=1.0, scale=1.0)
            hps = pp.tile([128, CLOC], F32, name="hps", tag="hps")
            nc.tensor.matmul(hps[:, 0:512], w1a[:], cm[:, 0:512], start=True, stop=False)
            nc.tensor.matmul(hps[:, 0:512], w1b[:], cs[:, 0:512], start=False, stop=True)
            nc.tensor.matmul(hps[:, 512:CLOC], w1a[:], cm[:, 512:CLOC], start=True, stop=False)
            nc.tensor.matmul(hps[:, 512:CLOC], w1b[:], cs[:, 512:CLOC], start=False, stop=True)
            hb = pool.tile([128, CLOC], BF, name="hb", tag="hb")
            nc.scalar.activation(hb[:], hps[:], AF.Exp, bias=fb1[:], scale=1.0)
            nc.scalar.activation(hb[:], hb[:], AF.Ln, bias=1.0, scale=1.0)
            ops = pp.tile([1, CLOC], F32, name="ops", tag="ops")
            nc.tensor.matmul(ops[:, 0:512], wo[:], hb[:, 0:512], start=True, stop=True)
            nc.tensor.matmul(ops[:, 512:CLOC], wo[:], hb[:, 512:CLOC], start=True, stop=True)
            ot = pool.tile([1, CLOC], F32, name="ot", tag="ot")
            nc.vector.tensor_copy(ot[:], ops[:])
            nc.sync.dma_start(tout[:], ot[:])
    nc.compile()
    return nc


def kernel(orig_atom_fea, nbr_fea, nbr_fea_idx, segment_ids,
           emb_W, emb_b, msg_W, msg_b, bn2_g, bn2_b, bn1_g, bn1_b,
           fc1_W, fc1_b, out_W, out_b):
    f32 = np.float32
    if "E" not in _cache:
        _cache["E"] = _build_embed()
        _cache["L"] = _build_layer()
        _cache["P"] = _build_pool()
    ncE, ncL, ncP = _cache["E"], _cache["L"], _cache["P"]
    cores = list(range(NC))

    origT = np.zeros((ORIG, NC, APAD), BF16)
    origT[:, :, :NLOC] = orig_atom_fea.astype(f32).T.reshape(ORIG, NC, NLOC)
    nbrT = nbr_fea.astype(f32).reshape(NC, COLS, NBR).transpose(0, 2, 1).astype(BF16)
    idx = nbr_fea_idx.astype(np.int64).reshape(NC, COLS)
    selfidx = np.repeat(np.arange(N, dtype=np.int64), M).reshape(NC, COLS)

    emb_W = np.asarray(emb_W, f32); emb_b = np.asarray(emb_b, f32)
    msg_W = np.asarray(msg_W, f32); msg_b = np.asarray(msg_b, f32)
    bn2_g = np.asarray(bn2_g, f32); bn2_b = np.asarray(bn2_b, f32)
    bn1_g = np.asarray(bn1_g, f32); bn1_b = np.asarray(bn1_b, f32)
    fc1_W = np.asarray(fc1_W, f32); fc1_b = np.asarray(fc1_b, f32)
    out_W = np.asarray(out_W, f32); out_b = np.asarray(out_b, f32)

    maps = [dict(oa=origT[0:128, c], ob=origT[128:200, c],
                 wa=emb_W[0:128].astype(BF16), wb=emb_W[128:200].astype(BF16),
                 eb=emb_b.reshape(64, 1)) for c in cores]
    res = run_bass_kernel_spmd(ncE, maps, core_ids=cores)
    atom = np.stack([r["atom"] for r in res.results])      # [NC, 64, APAD] f32

    for i in range(NG):
        full = np.concatenate([a[:, :NLOC] for a in atom], axis=1)  # [64, N]
        gs = np.empty((NC, 128, COLS), BF16)
        for c in cores:
            gs[c, 64:128] = full[:, idx[c]].astype(BF16)   # nbr rows -> Ws slot? no: order
            gs[c, 0:64] = full[:, selfidx[c]].astype(BF16)
        # wsn rows: [W_self(0:64); W_nbr(64:128)] to match gs packing
        wsn = np.concatenate([msg_W[i][0:AFL], msg_W[i][AFL:2 * AFL]], axis=0).astype(BF16)
        we = msg_W[i][2 * AFL:].astype(BF16)
        maps = [dict(gs=gs[c], nb=nbrT[c], atin=atom[c],
                     wsn=wsn, we=we,
                     b2=msg_b[i].reshape(128, 1),
                     g2=bn2_g[i].reshape(128, 1), bb2=bn2_b[i].reshape(128, 1),
                     g1=bn1_g[i].reshape(64, 1), bb1=bn1_b[i].reshape(64, 1))
                for c in cores]
        res = run_bass_kernel_spmd(ncL, maps, core_ids=cores)
        atom = np.stack([r["atout"] for r in res.results])

    maps = [dict(atin=atom[c],
                 w1a=fc1_W[0:64].astype(BF16), w1b=fc1_W[64:128].astype(BF16),
                 fb1=fc1_b.reshape(128, 1), wo=out_W.astype(BF16))
            for c in cores]
    res = run_bass_kernel_spmd(ncP, maps, core_ids=cores)
    out = np.concatenate([r["out"][0] for r in res.results])
    return (out + out_b[0]).reshape(N0, 1).astype(np.float32)

